# revision 29
# baseline (speedup 1.0000x reference)
"""Depthwise causal conv1d kernel for Trainium2 (8 NeuronCores, SPMD).

Problem: x [B=8, T=4096, C=512] f32, weight [C=512, K=4] f32.
out[b, t, c] = sum_k weight[c, k] * x[b, t - 3 + k, c]   (causal, zero-pad)

Strategy:
  - Data-parallel over batch: core b handles x[b].
  - Host-side layout: channels-first x[b].T with 3 leading zero columns
    -> [C=512, T+3=4099], cast fp16 (halves HBM traffic, ~2^-11 element
    error); accumulation stays fp32 in PSUM.
  - TensorE computes taps 0..2 as accumulating diag-matmuls (3 x 512-row
    matmuls per 512-col half of a [128,1024] PSUM pair-tile). Tap 3 is
    fused into the PSUM->SBUF drain on the Vector engine
    (scalar_tensor_tensor: out_f16 = x_shift3*w3 + psum), which costs
    the same as the plain cast it replaces and cuts PE work 25%
    (128 -> 96 matmuls). The pair drains hug the PE stream; the final
    pair drains as 2 x 512 passes, each shipped immediately, to
    shorten the kernel tail.
  - PE DVFS: the PE runs at ~0.83ns/row for ~4.5us after any idle gap
    before reaching 0.42ns/row, and the ramp restarts on every gap. A
    warmup bridge of dummy matmuls (reading a raw, never-written SBUF
    scratch that the Tile scheduler has no dependency edges for) starts
    right at the PE preamble end (~6.8us) and spans until the first x
    piece's DMA completion semaphore is reliably visible (~12.5us):
    9 x 512-wide then 20 x 128-wide (fine-grained tail), in
    accumulation groups of 3 alternating PSUM banks (standalone
    start+stop matmuls on one bank serialize on the PSUM write port).
    Real matmuls then run at the full rate from the first instruction.
  - DMA completion semaphores cost ~2us each and serialize per issuing
    queue, so DMAs are ordered by consumer need: x pieces get the
    Scalar DGE queue to themselves (first piece first); the Sync queue
    carries wd0 (gates the first matmul) before wcol/wcol32, then the
    outputs. chunk0's diag stationary (wd0) is prebuilt on the host so
    no GpSimd expansion gates the first matmul. All queues share the
    16 physical DMA engines (~380 GB/s/core): 8.4 MB of traffic
    ~= 22 us, overlapped with the ~21 us PE stream.
"""

import numpy as np

B, T, C, K = 8, 4096, 512, 4
P = 128  # partitions
NCHUNK = C // P  # 4 channel chunks
TJ = 512  # matmul moving width; one PSUM bank
TT = 4 * TJ  # 2048: PSUM tile width (4 banks)
NT = T // TT  # 2 PSUM tiles per chunk
TP = T + K - 1  # padded time = 4099
NW = NCHUNK * K  # 16 (chunk, tap) columns in the weight table
KM = K - 1  # taps done on TensorE (0..2); tap 3 fused on DVE
NWARM_BIG = 9  # 512-wide warmup matmuls (~427ns each at mid p-state)
NWARM_SMALL = 20  # 128-wide warmup matmuls (~107ns each) for a
# fine-grained landing just past the first x piece's worst-case arrival

_compiled = None


def _build():
    import contextlib

    import concourse.bacc as bacc
    import concourse.bass as bass
    import concourse.mybir as mybir
    from concourse.tile import TileContext

    f32 = mybir.dt.float32
    f16 = mybir.dt.float16
    nc = bacc.Bacc(enable_partition_id=False)

    wt32_d = nc.declare_dram_parameter("wt32", [P, NW], f32, isOutput=False)
    wd0_d = nc.declare_dram_parameter("wd0", [P, KM * P], f16, isOutput=False)
    wda_d = nc.declare_dram_parameter(
        "wda", [P, (NCHUNK - 1) * KM * P], f16, isOutput=False
    )
    xw_d = nc.declare_dram_parameter("xw", [P, NCHUNK * TP], f16, isOutput=False)
    out_d = nc.declare_dram_parameter("out", [C, T], f16, isOutput=True)

    ctx = contextlib.ExitStack()
    # Raw (non-Tile) SBUF scratch for the PE warmup: never written, so
    # the warmup matmuls have no dependencies at all and issue the
    # moment the PE preamble ends. Garbage input is fine — the PSUM
    # results are overwritten by the first real start=True matmul.
    scr = ctx.enter_context(nc.sbuf_tensor([P, TJ], f16))

    with TileContext(nc) as tc:
        with (
            tc.tile_pool(name="xpool", bufs=1) as xpool,
            tc.tile_pool(name="wpool", bufs=1) as wpool,
            tc.tile_pool(name="opool", bufs=4) as opool,
            tc.tile_pool(name="ppool", bufs=4, space="PSUM") as ppool,
        ):
            # small weight DMAs on the Sync queue (its first output DMA
            # is much later). Queue completion semaphores fire in issue
            # order with multi-us spacing, so wd0 — which gates the very
            # first real matmul — must be the FIRST DMA on this queue;
            # wcol gates only chunk1's expansion (~17us) and wcol32 only
            # the first DVE drain (~15us).
            wd0 = wpool.tile([P, KM * P], f16, name="wd0", tag="wd0")
            nc.sync.dma_start(out=wd0, in_=wd0_d[:, :])
            wcol32 = wpool.tile([P, NW], f32, name="wcol32", tag="wcol32")
            nc.sync.dma_start(out=wcol32, in_=wt32_d[:, :])
            wda = wpool.tile(
                [P, (NCHUNK - 1) * KM * P], f16, name="wda", tag="wda"
            )
            nc.sync.dma_start(out=wda, in_=wda_d[:, :])

            # x loads get the Scalar DGE queue to themselves; chunk0 in
            # two PSUM-tile-sized pieces (the warmup bridge outlasts the
            # first piece's DMA semaphore, so bigger pieces are free)
            xts = []
            xt0 = xpool.tile([P, TP], f16, name="xt0", tag="xt0")
            h0 = 2 * TJ + K - 1  # 1027: pair-tile 0 + tap halo
            h1 = 4 * TJ + K - 1  # 2051: pair-tile 1
            nc.scalar.dma_start(out=xt0[:, :h0], in_=xw_d[:, 0:h0])
            nc.scalar.dma_start(out=xt0[:, h0:h1], in_=xw_d[:, h0:h1])
            nc.scalar.dma_start(out=xt0[:, h1:], in_=xw_d[:, h1:TP])
            xts.append(xt0)
            for c in range(1, NCHUNK):
                xt = xpool.tile([P, TP], f16, name=f"xt{c}", tag=f"xt{c}")
                nc.scalar.dma_start(out=xt, in_=xw_d[:, c * TP : (c + 1) * TP])
                xts.append(xt)

            # all diag stationaries come prebuilt from the host — no
            # GpSimd expansion, no wcol semaphore on any matmul's path
            wts = [wd0] + [
                wda[:, (c - 1) * KM * P : c * KM * P] for c in range(1, NCHUNK)
            ]

            # PE warmup bridge (see module docstring)
            ptw = ppool.tile([P, 2 * TJ], f32, name="ptw", tag="pt")
            for i in range(NWARM_BIG + NWARM_SMALL):
                w = TJ if i < NWARM_BIG else P
                half = (i // 3) % 2
                nc.tensor.matmul(
                    ptw[:, half * TJ : half * TJ + w],
                    scr[:, :P],
                    scr[:, :w],
                    start=(i % 3 == 0),
                    stop=(i % 3 == 2 or i == NWARM_BIG + NWARM_SMALL - 1),
                )

            TJ2 = 2 * TJ  # pair tile: one 2-bank PSUM tile, one DVE drain
            NP2 = T // TJ2  # 4 pairs per chunk
            for chunk in range(NCHUNK):
                xv = xts[chunk]
                wt = wts[chunk]
                w3 = wcol32[:, chunk * K + KM : chunk * K + KM + 1]  # [128,1]
                ot = opool.tile([P, T], f16, tag="ot")
                for jj in range(NP2):
                    base = jj * TJ2
                    pt = ppool.tile([P, TJ2], f32, name="pt", tag="pt")
                    for half in range(2):
                        j = base + half * TJ
                        for k in range(KM):
                            nc.tensor.matmul(
                                pt[:, half * TJ : (half + 1) * TJ],
                                wt[:, k * P : (k + 1) * P],
                                xv[:, j + k : j + k + TJ],
                                start=(k == 0),
                                stop=(k == KM - 1),
                            )
                    # fused tap-3 + downcast drain: ot = (x_s3 * w3) + pt.
                    # The very last pair drains as 2 x 512 passes with
                    # per-piece shipping to shorten the kernel tail.
                    last = chunk == NCHUNK - 1
                    tail = last and jj == NP2 - 1
                    nsub = 2 if tail else 1
                    sub_w = TJ2 // nsub
                    for s in range(nsub):
                        lo = base + s * sub_w
                        nc.vector.scalar_tensor_tensor(
                            out=ot[:, lo : lo + sub_w],
                            in0=xv[:, lo + K - 1 : lo + K - 1 + sub_w],
                            scalar=w3,
                            in1=pt[:, s * sub_w : (s + 1) * sub_w],
                            op0=mybir.AluOpType.mult,
                            op1=mybir.AluOpType.add,
                        )
                        if last:
                            nc.scalar.dma_start(
                                out=out_d[chunk * P : (chunk + 1) * P, lo : lo + sub_w],
                                in_=ot[:, lo : lo + sub_w],
                            )
                    # chunks 0..2 ship per half-chunk
                    if not last and jj % 2 == 1:
                        half_c = (jj - 1) * TJ2
                        nc.sync.dma_start(
                            out=out_d[
                                chunk * P : (chunk + 1) * P,
                                half_c : half_c + 2 * TJ2,
                            ],
                            in_=ot[:, half_c : half_c + 2 * TJ2],
                        )

    nc.compile()
    ctx.close()
    return nc


def _prep_inputs(x: np.ndarray, weight: np.ndarray):
    # wcol[p, chunk*K + k] = weight[chunk*P + p, k]
    wcol = np.ascontiguousarray(
        weight.reshape(NCHUNK, P, K).transpose(1, 0, 2).reshape(P, NW)
    ).astype(np.float16)
    # chunk0's diag stationary prebuilt: wd0[p, k*P + p] = weight[p, k]
    wd0 = np.zeros((P, KM * P), dtype=np.float16)
    for k in range(KM):
        wd0[np.arange(P), k * P + np.arange(P)] = weight[:P, k].astype(np.float16)
    wda = np.zeros((P, (NCHUNK - 1) * KM * P), dtype=np.float16)
    for c in range(1, NCHUNK):
        for k in range(KM):
            wda[
                np.arange(P), (c - 1) * KM * P + k * P + np.arange(P)
            ] = weight[c * P : (c + 1) * P, k].astype(np.float16)
    xs = []
    for b in range(B):
        xp = np.zeros((C, TP), dtype=np.float32)
        xp[:, K - 1 :] = x[b].T  # [512, 4099], 3 leading zeros
        xw = np.ascontiguousarray(
            xp.reshape(NCHUNK, P, TP).transpose(1, 0, 2).reshape(P, NCHUNK * TP)
        ).astype(np.float16)
        xs.append(xw)
    wcol32 = np.ascontiguousarray(
        weight.reshape(NCHUNK, P, K).transpose(1, 0, 2).reshape(P, NW)
    ).astype(np.float32)
    return xs, wd0, wda, wcol32


def _ensure_axon_hooks():
    """This image's antenv package lacks axon_hooks; synthesize it so a
    trace=True / BASS_TRACE run of run_bass_kernel_spmd can profile
    instead of crashing on import."""
    import sys
    import types

    if "antenv.axon_hooks" in sys.modules:
        return
    mod = types.ModuleType("antenv.axon_hooks")
    state = {"hook": None}
    mod.set_axon_ntff_profile_hook = lambda h: state.__setitem__("hook", h)
    mod.get_axon_ntff_profile_hook = lambda: state["hook"]
    sys.modules["antenv.axon_hooks"] = mod
    try:
        if "/root/.axon_site" not in sys.path:
            sys.path.insert(0, "/root/.axon_site")
        from trn_agent_boot.trn_boot import _ntff_profile_via_ctypes

        mod.set_axon_ntff_profile_hook(
            _ntff_profile_via_ctypes("/opt/axon/libaxon_pjrt.so")
        )
    except Exception:
        pass  # hook stays None; concourse degrades to no-trace


def kernel(x: np.ndarray, weight: np.ndarray) -> np.ndarray:
    global _compiled
    _ensure_axon_hooks()
    from concourse import bass_utils

    x = np.ascontiguousarray(x, dtype=np.float32)
    weight = np.ascontiguousarray(weight, dtype=np.float32)

    if _compiled is None:
        _compiled = _build()
    nc = _compiled

    xs, wd0, wda, wcol32 = _prep_inputs(x, weight)
    in_maps = [
        {"xw": xs[b], "wd0": wd0, "wda": wda, "wt32": wcol32} for b in range(B)
    ]
    res = bass_utils.run_bass_kernel_spmd(nc, in_maps, core_ids=list(range(B)))

    out = np.empty((B, T, C), dtype=np.float32)
    for b in range(B):
        out[b] = np.asarray(res.results[b]["out"]).astype(np.float32).T
    return out


# revision 31
# speedup vs baseline: 1.1947x; 1.1947x over previous
"""Depthwise causal conv1d kernel for Trainium2 (8 NeuronCores, SPMD).

Problem: x [B=8, T=4096, C=512] f32, weight [C=512, K=4] f32.
out[b, t, c] = sum_k weight[c, k] * x[b, t - 3 + k, c]   (causal, zero-pad)

Strategy:
  - Data-parallel over batch: core b handles x[b].
  - Host-side layout: channels-first x[b].T with 3 leading zero columns
    -> [C=512, T+3=4099], cast fp16 (halves HBM traffic, ~2^-11 element
    error); accumulation stays fp32 in PSUM.
  - TensorE computes taps 0..2 as accumulating diag-matmuls (3 x 512-row
    matmuls per 512-col half of a [128,1024] PSUM pair-tile). Tap 3 is
    fused into the PSUM->SBUF drain on the Vector engine
    (scalar_tensor_tensor: out_f16 = x_shift3*w3 + psum), which costs
    the same as the plain cast it replaces and cuts PE work 25%
    (128 -> 96 matmuls). The pair drains hug the PE stream; the final
    pair drains as 2 x 512 passes, each shipped immediately, to
    shorten the kernel tail.
  - PE DVFS: the PE runs at ~0.83ns/row for ~4.5us after any idle gap
    before reaching 0.42ns/row, and the ramp restarts on every gap. A
    warmup bridge of dummy matmuls (reading a raw, never-written SBUF
    scratch that the Tile scheduler has no dependency edges for) starts
    right at the PE preamble end (~6.8us) and spans until the first x
    piece's DMA completion semaphore is reliably visible (~12.5us):
    9 x 512-wide then 20 x 128-wide (fine-grained tail), in
    accumulation groups of 3 alternating PSUM banks (standalone
    start+stop matmuls on one bank serialize on the PSUM write port).
    Real matmuls then run at the full rate from the first instruction.
  - DMA completion semaphores cost ~2us each and serialize per issuing
    queue, so DMAs are ordered by consumer need: x pieces get the
    Scalar DGE queue to themselves (first piece first); the Sync queue
    carries wd0 (gates the first matmul) before wcol/wcol32, then the
    outputs. chunk0's diag stationary (wd0) is prebuilt on the host so
    no GpSimd expansion gates the first matmul. All queues share the
    16 physical DMA engines (~380 GB/s/core): 8.4 MB of traffic
    ~= 22 us, overlapped with the ~21 us PE stream.
"""

import numpy as np

B, T, C, K = 8, 4096, 512, 4
P = 128  # partitions
NCHUNK = C // P  # 4 channel chunks
TJ = 512  # matmul moving width; one PSUM bank
TT = 4 * TJ  # 2048: PSUM tile width (4 banks)
NT = T // TT  # 2 PSUM tiles per chunk
TP = T + K - 1  # padded time = 4099
NW = NCHUNK * K  # 16 (chunk, tap) columns in the weight table
KM = K - 1  # taps done on TensorE (0..2); tap 3 fused on DVE
NWARM_BIG = 9  # 512-wide warmup matmuls (~427ns each at mid p-state)
NWARM_SMALL = 20  # 128-wide warmup matmuls (~107ns each) for a
# fine-grained landing just past the first x piece's worst-case arrival

_compiled = None


def _build():
    import contextlib

    import concourse.bacc as bacc
    import concourse.bass as bass
    import concourse.mybir as mybir
    from concourse.tile import TileContext

    f32 = mybir.dt.float32
    f16 = mybir.dt.float16
    nc = bacc.Bacc(enable_partition_id=False)

    wt32_d = nc.declare_dram_parameter("wt32", [P, NW], f32, isOutput=False)
    wd0_d = nc.declare_dram_parameter("wd0", [P, KM * P], f16, isOutput=False)
    wda_d = nc.declare_dram_parameter(
        "wda", [P, (NCHUNK - 1) * KM * P], f16, isOutput=False
    )
    xw_d = nc.declare_dram_parameter("xw", [P, NCHUNK * TP], f16, isOutput=False)
    out_d = nc.declare_dram_parameter("out", [C, T], f16, isOutput=True)

    ctx = contextlib.ExitStack()
    # Raw (non-Tile) SBUF scratch for the PE warmup: never written, so
    # the warmup matmuls have no dependencies at all and issue the
    # moment the PE preamble ends. Garbage input is fine — the PSUM
    # results are overwritten by the first real start=True matmul.
    scr = ctx.enter_context(nc.sbuf_tensor([P, TJ], f16))

    with TileContext(nc) as tc:
        with (
            tc.tile_pool(name="xpool", bufs=1) as xpool,
            tc.tile_pool(name="wpool", bufs=1) as wpool,
            tc.tile_pool(name="opool", bufs=4) as opool,
            tc.tile_pool(name="ppool", bufs=4, space="PSUM") as ppool,
        ):
            # small weight DMAs on the Sync queue (its first output DMA
            # is much later). Queue completion semaphores fire in issue
            # order with multi-us spacing, so wd0 — which gates the very
            # first real matmul — must be the FIRST DMA on this queue;
            # wcol gates only chunk1's expansion (~17us) and wcol32 only
            # the first DVE drain (~15us).
            wd0 = wpool.tile([P, KM * P], f16, name="wd0", tag="wd0")
            nc.sync.dma_start(out=wd0, in_=wd0_d[:, :])
            wcol32 = wpool.tile([P, NW], f32, name="wcol32", tag="wcol32")
            nc.sync.dma_start(out=wcol32, in_=wt32_d[:, :])
            wda = wpool.tile(
                [P, (NCHUNK - 1) * KM * P], f16, name="wda", tag="wda"
            )
            nc.sync.dma_start(out=wda, in_=wda_d[:, :])

            # x loads get the Scalar DGE queue to themselves; chunk0 in
            # two PSUM-tile-sized pieces (the warmup bridge outlasts the
            # first piece's DMA semaphore, so bigger pieces are free)
            xts = []
            xt0 = xpool.tile([P, TP], f16, name="xt0", tag="xt0")
            h0 = 2 * TJ + K - 1  # 1027: pair-tile 0 + tap halo
            h1 = 4 * TJ + K - 1  # 2051: pair-tile 1
            nc.scalar.dma_start(out=xt0[:, :h0], in_=xw_d[:, 0:h0])
            nc.scalar.dma_start(out=xt0[:, h0:h1], in_=xw_d[:, h0:h1])
            nc.scalar.dma_start(out=xt0[:, h1:], in_=xw_d[:, h1:TP])
            xts.append(xt0)
            for c in range(1, NCHUNK):
                xt = xpool.tile([P, TP], f16, name=f"xt{c}", tag=f"xt{c}")
                nc.scalar.dma_start(out=xt, in_=xw_d[:, c * TP : (c + 1) * TP])
                xts.append(xt)

            # all diag stationaries come prebuilt from the host: no
            # GpSimd expansion, so no wcol-semaphore -> affine-select
            # chain on the chunk-transition matmuls' critical path (that
            # chain cost a recurring 0.3-0.9us PE stall, and any PE stall
            # also restarts the DVFS ramp)
            wts = [wd0] + [
                wda[:, (c - 1) * KM * P : c * KM * P] for c in range(1, NCHUNK)
            ]

            # PE warmup bridge (see module docstring)
            ptw = ppool.tile([P, 2 * TJ], f32, name="ptw", tag="pt")
            for i in range(NWARM_BIG + NWARM_SMALL):
                w = TJ if i < NWARM_BIG else P
                half = (i // 3) % 2
                nc.tensor.matmul(
                    ptw[:, half * TJ : half * TJ + w],
                    scr[:, :P],
                    scr[:, :w],
                    start=(i % 3 == 0),
                    stop=(i % 3 == 2 or i == NWARM_BIG + NWARM_SMALL - 1),
                )

            TJ2 = 2 * TJ  # pair tile: one 2-bank PSUM tile, one DVE drain
            NP2 = T // TJ2  # 4 pairs per chunk
            for chunk in range(NCHUNK):
                xv = xts[chunk]
                wt = wts[chunk]
                w3 = wcol32[:, chunk * K + KM : chunk * K + KM + 1]  # [128,1]
                ot = opool.tile([P, T], f16, tag="ot")
                for jj in range(NP2):
                    base = jj * TJ2
                    pt = ppool.tile([P, TJ2], f32, name="pt", tag="pt")
                    for half in range(2):
                        j = base + half * TJ
                        for k in range(KM):
                            nc.tensor.matmul(
                                pt[:, half * TJ : (half + 1) * TJ],
                                wt[:, k * P : (k + 1) * P],
                                xv[:, j + k : j + k + TJ],
                                start=(k == 0),
                                stop=(k == KM - 1),
                            )
                    # fused tap-3 + downcast drain: ot = (x_s3 * w3) + pt.
                    # The very last pair drains as 2 x 512 passes with
                    # per-piece shipping to shorten the kernel tail.
                    last = chunk == NCHUNK - 1
                    tail = last and jj == NP2 - 1
                    nsub = 2 if tail else 1
                    sub_w = TJ2 // nsub
                    for s in range(nsub):
                        lo = base + s * sub_w
                        nc.vector.scalar_tensor_tensor(
                            out=ot[:, lo : lo + sub_w],
                            in0=xv[:, lo + K - 1 : lo + K - 1 + sub_w],
                            scalar=w3,
                            in1=pt[:, s * sub_w : (s + 1) * sub_w],
                            op0=mybir.AluOpType.mult,
                            op1=mybir.AluOpType.add,
                        )
                        if last:
                            nc.sync.dma_start(
                                out=out_d[chunk * P : (chunk + 1) * P, lo : lo + sub_w],
                                in_=ot[:, lo : lo + sub_w],
                            )
                    # chunks 0..2 ship per half-chunk
                    if not last and jj % 2 == 1:
                        half_c = (jj - 1) * TJ2
                        nc.sync.dma_start(
                            out=out_d[
                                chunk * P : (chunk + 1) * P,
                                half_c : half_c + 2 * TJ2,
                            ],
                            in_=ot[:, half_c : half_c + 2 * TJ2],
                        )

    nc.compile()
    ctx.close()
    return nc


def _prep_inputs(x: np.ndarray, weight: np.ndarray):
    # wcol[p, chunk*K + k] = weight[chunk*P + p, k]
    wcol = np.ascontiguousarray(
        weight.reshape(NCHUNK, P, K).transpose(1, 0, 2).reshape(P, NW)
    ).astype(np.float16)
    # chunk0's diag stationary prebuilt: wd0[p, k*P + p] = weight[p, k]
    wd0 = np.zeros((P, KM * P), dtype=np.float16)
    for k in range(KM):
        wd0[np.arange(P), k * P + np.arange(P)] = weight[:P, k].astype(np.float16)
    wda = np.zeros((P, (NCHUNK - 1) * KM * P), dtype=np.float16)
    for c in range(1, NCHUNK):
        for k in range(KM):
            wda[
                np.arange(P), (c - 1) * KM * P + k * P + np.arange(P)
            ] = weight[c * P : (c + 1) * P, k].astype(np.float16)
    xs = []
    for b in range(B):
        xp = np.zeros((C, TP), dtype=np.float32)
        xp[:, K - 1 :] = x[b].T  # [512, 4099], 3 leading zeros
        xw = np.ascontiguousarray(
            xp.reshape(NCHUNK, P, TP).transpose(1, 0, 2).reshape(P, NCHUNK * TP)
        ).astype(np.float16)
        xs.append(xw)
    wcol32 = np.ascontiguousarray(
        weight.reshape(NCHUNK, P, K).transpose(1, 0, 2).reshape(P, NW)
    ).astype(np.float32)
    return xs, wd0, wda, wcol32


def _ensure_axon_hooks():
    """This image's antenv package lacks axon_hooks; synthesize it so a
    trace=True / BASS_TRACE run of run_bass_kernel_spmd can profile
    instead of crashing on import."""
    import sys
    import types

    if "antenv.axon_hooks" in sys.modules:
        return
    mod = types.ModuleType("antenv.axon_hooks")
    state = {"hook": None}
    mod.set_axon_ntff_profile_hook = lambda h: state.__setitem__("hook", h)
    mod.get_axon_ntff_profile_hook = lambda: state["hook"]
    sys.modules["antenv.axon_hooks"] = mod
    try:
        if "/root/.axon_site" not in sys.path:
            sys.path.insert(0, "/root/.axon_site")
        from trn_agent_boot.trn_boot import _ntff_profile_via_ctypes

        mod.set_axon_ntff_profile_hook(
            _ntff_profile_via_ctypes("/opt/axon/libaxon_pjrt.so")
        )
    except Exception:
        pass  # hook stays None; concourse degrades to no-trace


def kernel(x: np.ndarray, weight: np.ndarray) -> np.ndarray:
    global _compiled
    _ensure_axon_hooks()
    from concourse import bass_utils

    x = np.ascontiguousarray(x, dtype=np.float32)
    weight = np.ascontiguousarray(weight, dtype=np.float32)

    if _compiled is None:
        _compiled = _build()
    nc = _compiled

    xs, wd0, wda, wcol32 = _prep_inputs(x, weight)
    in_maps = [
        {"xw": xs[b], "wd0": wd0, "wda": wda, "wt32": wcol32} for b in range(B)
    ]
    res = bass_utils.run_bass_kernel_spmd(nc, in_maps, core_ids=list(range(B)))

    out = np.empty((B, T, C), dtype=np.float32)
    for b in range(B):
        out[b] = np.asarray(res.results[b]["out"]).astype(np.float32).T
    return out
